# revision 8
# baseline (speedup 1.0000x reference)
"""BsplineKAN fused kernel for Trainium2 (8 NeuronCores, batch-sharded).

Math (per reference):
  basis = truncated in-place Cox-de Boor, degree 3, K=11 uniform knots on [0,1]
  out   = LN(einsum('bik,oik->bo', basis, cp) + x @ W.T + b) * gamma + beta

Closed form used here (u = 11*x, s_m = relu(u - m)):
  basis_k (k=0..7) = (1/6) * [s_k^3 - 4 s_{k+1}^3 + 6 s_{k+2}^3 - 4 s_{k+3}^3 + s_{k+4}^3]
  basis_8  = (1/2) * [s_8^2 - 3 s_9^2 + 3 s_10^2]
  basis_9  = s_9 - 2 s_10
  basis_10 = (sign(u - 10) + 1) / 2
The linear layer is fused as a 12th basis column (feature = x, weights = W),
the +1/2 constant of basis_10 and the bias b are folded into a single K=1
ones-row matmul. Scale factors (1/6, 1/2) are folded into the control-point
matrix on the host. The big contraction (K = 12*1024) runs on the PE in bf16;
the basis is combined on-device in fp32 (the relu^3 terms reach ~1300 while
basis values are <1, so pre-combine bf16 quantization would be catastrophic).
"""

import functools
import numpy as np
import ml_dtypes

BATCH = 16384
INF = 1024
OUTF = 1024
NCORES = 8
BC = BATCH // NCORES        # 2048 batch rows per core
BMS = 512                   # batch-macro size (basis slice width)
NBM = BC // BMS             # 4 macros
IB = INF // 128             # 8 i-blocks
CPI = 12                    # feature rows per i (11 spline cols + x)
NCHUNK = IB * CPI           # 96 contraction chunks of 128
EPS = 1e-5


@functools.lru_cache(maxsize=1)
def _build_nc():
    import concourse.bass as bass
    import concourse.mybir as mybir
    import concourse.tile as tile
    from concourse import bacc

    f32 = mybir.dt.float32
    bf16 = mybir.dt.bfloat16
    AF = mybir.ActivationFunctionType
    OP = mybir.AluOpType

    nc = bacc.Bacc("TRN2", target_bir_lowering=False, debug=False)
    xT = nc.dram_tensor("xT", [INF, BC], f32, kind="ExternalInput").ap()
    cpb = nc.dram_tensor("cpb", [NCHUNK * 128, OUTF], bf16, kind="ExternalInput").ap()
    brow = nc.dram_tensor("brow", [1, OUTF], f32, kind="ExternalInput").ap()
    gam = nc.dram_tensor("gam", [1, OUTF], f32, kind="ExternalInput").ap()
    bet = nc.dram_tensor("bet", [1, OUTF], f32, kind="ExternalInput").ap()
    out_d = nc.dram_tensor("out", [BC, OUTF], f32, kind="ExternalOutput").ap()

    with tile.TileContext(nc) as tc:
        from contextlib import ExitStack
        with ExitStack() as ctx:
            ep = ctx.enter_context
            consts = ep(tc.tile_pool(name="consts", bufs=1))
            xpool = ep(tc.tile_pool(name="xp", bufs=2))
            spool = ep(tc.tile_pool(name="sp", bufs=1))
            s2pool = ep(tc.tile_pool(name="s2p", bufs=4))
            s3pool = ep(tc.tile_pool(name="s3p", bufs=1))
            tpool = ep(tc.tile_pool(name="tp", bufs=6))
            bpool = ep(tc.tile_pool(name="bp", bufs=2))
            wpool = ep(tc.tile_pool(name="wp", bufs=3))
            zpool = ep(tc.tile_pool(name="zp", bufs=2))
            stpool = ep(tc.tile_pool(name="stp", bufs=2))
            ypool = ep(tc.tile_pool(name="yp", bufs=2))
            ppool = ep(tc.tile_pool(name="pp", bufs=8, space="PSUM"))

            gamma_t = consts.tile([128, OUTF], f32)
            nc.sync.dma_start(out=gamma_t, in_=gam.partition_broadcast(128))
            beta_t = consts.tile([128, OUTF], f32)
            nc.sync.dma_start(out=beta_t, in_=bet.partition_broadcast(128))
            brow_t = consts.tile([1, OUTF], f32)
            nc.sync.dma_start(out=brow_t, in_=brow)
            ones_t = consts.tile([1, 128], f32)
            nc.vector.memset(ones_t, 1.0)
            # col 0: eps for LN; cols 1..11: -m ACT bias constants
            mconst = consts.tile([128, 12], f32)
            nc.vector.memset(mconst[:, 0:1], EPS)
            for m in range(11):
                nc.vector.memset(mconst[:, m + 1:m + 2], -float(m))

            for bm in range(NBM):
                psums = [[ppool.tile([128, 512], f32, name="psum", tag="psum")
                          for _ in range(2)]
                         for _ in range(4)]
                for ib in range(IB):
                    xt = xpool.tile([128, BMS], f32)
                    nc.sync.dma_start(
                        out=xt, in_=xT[ib * 128:(ib + 1) * 128,
                                       bm * BMS:(bm + 1) * BMS])
                    # s[:, m, :] = relu(11*x - m), fp32
                    st = spool.tile([128, 11, BMS], f32)
                    for m in range(11):
                        nc.scalar.activation(out=st[:, m, :], in_=xt,
                                             func=AF.Relu,
                                             bias=mconst[:, m + 1:m + 2],
                                             scale=11.0)
                    # s3[:, m, :] = s^3 via rotating s2 tiles
                    s3t = s3pool.tile([128, 11, BMS], f32)
                    for m in range(11):
                        s2m = s2pool.tile([128, BMS], f32, name="s2m")
                        nc.scalar.activation(out=s2m, in_=st[:, m, :],
                                             func=AF.Square)
                        nc.vector.tensor_mul(s3t[:, m, :], s2m, st[:, m, :])

                    bsl = bpool.tile([128, CPI, BMS], bf16)
                    # cubic cols: 4th difference of s^3 (1/6 folded in cpb)
                    for k in range(8):
                        t1 = tpool.tile([128, BMS], f32, name="tt", tag="tt")
                        nc.vector.scalar_tensor_tensor(
                            out=t1, in0=s3t[:, k + 1, :], scalar=-4.0,
                            in1=s3t[:, k, :], op0=OP.mult, op1=OP.add)
                        t2 = tpool.tile([128, BMS], f32, name="tt", tag="tt")
                        nc.vector.scalar_tensor_tensor(
                            out=t2, in0=s3t[:, k + 2, :], scalar=6.0, in1=t1,
                            op0=OP.mult, op1=OP.add)
                        if k < 7:
                            t3 = tpool.tile([128, BMS], f32, name="tt",
                                            tag="tt")
                            nc.vector.scalar_tensor_tensor(
                                out=t3, in0=s3t[:, k + 3, :], scalar=-4.0,
                                in1=t2, op0=OP.mult, op1=OP.add)
                            nc.vector.scalar_tensor_tensor(
                                out=bsl[:, k, :], in0=s3t[:, k + 4, :],
                                scalar=1.0, in1=t3, op0=OP.mult, op1=OP.add)
                        else:
                            nc.vector.scalar_tensor_tensor(
                                out=bsl[:, k, :], in0=s3t[:, 10, :],
                                scalar=-4.0, in1=t2, op0=OP.mult, op1=OP.add)
                    # quadratic col 8 = s8^2 - 3 s9^2 + 3 s10^2 (1/2 folded)
                    q8 = tpool.tile([128, BMS], f32, name="tt", tag="tt")
                    nc.vector.scalar_tensor_tensor(
                        out=q8, in0=st[:, 8, :], scalar=1.0, in1=st[:, 8, :],
                        op0=OP.mult, op1=OP.mult)
                    q9 = tpool.tile([128, BMS], f32, name="tt", tag="tt")
                    nc.vector.scalar_tensor_tensor(
                        out=q9, in0=st[:, 9, :], scalar=-3.0, in1=st[:, 9, :],
                        op0=OP.mult, op1=OP.mult)
                    q10 = tpool.tile([128, BMS], f32, name="tt", tag="tt")
                    nc.vector.scalar_tensor_tensor(
                        out=q10, in0=st[:, 10, :], scalar=3.0,
                        in1=st[:, 10, :], op0=OP.mult, op1=OP.mult)
                    qa = tpool.tile([128, BMS], f32, name="tt", tag="tt")
                    nc.vector.tensor_add(qa, q8, q9)
                    nc.vector.tensor_add(bsl[:, 8, :], qa, q10)
                    # linear col 9 = s9 - 2 s10
                    nc.vector.scalar_tensor_tensor(
                        out=bsl[:, 9, :], in0=st[:, 10, :], scalar=-2.0,
                        in1=st[:, 9, :], op0=OP.mult, op1=OP.add)
                    # step col 10 as sign (affine fold in cpb + brow)
                    nc.scalar.activation(out=bsl[:, 10, :], in_=xt,
                                         func=AF.Sign,
                                         bias=mconst[:, 11:12], scale=11.0)
                    # linear-layer feature: x itself
                    nc.vector.tensor_copy(out=bsl[:, 11, :], in_=xt)

                    for c in range(CPI):
                        chunk = ib * CPI + c
                        wt = wpool.tile([128, OUTF], bf16)
                        nc.sync.dma_start(
                            out=wt,
                            in_=cpb[chunk * 128:(chunk + 1) * 128, :])
                        first = (ib == 0 and c == 0)
                        for bs_i in range(4):
                            lhsT = bsl[:, c, bs_i * 128:(bs_i + 1) * 128]
                            for oh in range(2):
                                nc.tensor.matmul(
                                    psums[bs_i][oh], lhsT,
                                    wt[:, oh * 512:(oh + 1) * 512],
                                    start=first, stop=False)

                # bias row (b + 0.5*sum_i cp[:,i,10]) via ones-row matmul
                for bs_i in range(4):
                    for oh in range(2):
                        nc.tensor.matmul(
                            psums[bs_i][oh], ones_t,
                            brow_t[:, oh * 512:(oh + 1) * 512],
                            start=False, stop=True)

                # LayerNorm epilogue
                for bs_i in range(4):
                    z = zpool.tile([128, OUTF], f32)
                    nc.scalar.copy(z[:, 0:512], psums[bs_i][0])
                    nc.scalar.copy(z[:, 512:1024], psums[bs_i][1])
                    stats = stpool.tile([128, 2, 6], f32)
                    nc.vector.bn_stats(out=stats[:, 0, :], in_=z[:, 0:512])
                    nc.vector.bn_stats(out=stats[:, 1, :], in_=z[:, 512:1024])
                    mvsi = stpool.tile([128, 4], f32)
                    nc.vector.bn_aggr(out=mvsi[:, 0:2], in_=stats)
                    nc.scalar.activation(out=mvsi[:, 2:3], in_=mvsi[:, 1:2],
                                         func=AF.Sqrt, bias=mconst[:, 0:1])
                    nc.vector.reciprocal(out=mvsi[:, 3:4], in_=mvsi[:, 2:3])
                    y = ypool.tile([128, OUTF], f32)
                    nc.vector.tensor_scalar(
                        out=y, in0=z, scalar1=mvsi[:, 0:1],
                        scalar2=mvsi[:, 3:4],
                        op0=OP.subtract, op1=OP.mult)
                    nc.vector.tensor_mul(y, y, gamma_t)
                    nc.vector.tensor_add(y, y, beta_t)
                    row = bm * BMS + bs_i * 128
                    nc.sync.dma_start(out=out_d[row:row + 128, :], in_=y)

    nc.compile()
    return nc


def _host_prep(x, control_points, W, b):
    """Build per-core inputs. cpb row (k*1024+i) holds the weights for
    feature (k, i); scale factors folded in."""
    cp64 = control_points.astype(np.float64)
    blocks = []
    for k in range(12):
        if k < 8:
            blk = cp64[:, :, k].T / 6.0
        elif k == 8:
            blk = cp64[:, :, 8].T / 2.0
        elif k == 9:
            blk = cp64[:, :, 9].T
        elif k == 10:
            blk = cp64[:, :, 10].T / 2.0
        else:
            blk = W.astype(np.float64).T
        blocks.append(blk)
    # device chunk order: chunk = ib*12 + k  (i-block major, feature minor)
    kmaj = np.concatenate(blocks, axis=0).reshape(12, IB, 128, OUTF)
    cpb = np.ascontiguousarray(
        kmaj.transpose(1, 0, 2, 3).reshape(12 * INF, OUTF)
    ).astype(ml_dtypes.bfloat16)
    brow = (b.astype(np.float64)
            + 0.5 * cp64[:, :, 10].sum(axis=1)).astype(np.float32)[None, :]
    xT = np.ascontiguousarray(x.T)  # [INF, BATCH]
    return xT, cpb, brow


def kernel(x, control_points, W, b, gamma, beta):
    from concourse.bass_utils import run_bass_kernel_spmd

    xT, cpb, brow = _host_prep(x, control_points, W, b)
    gam = np.ascontiguousarray(gamma.astype(np.float32))[None, :]
    bet = np.ascontiguousarray(beta.astype(np.float32))[None, :]

    nc = _build_nc()
    in_maps = []
    for c in range(NCORES):
        in_maps.append({
            "xT": np.ascontiguousarray(xT[:, c * BC:(c + 1) * BC]),
            "cpb": cpb,
            "brow": brow,
            "gam": gam,
            "bet": bet,
        })
    res = run_bass_kernel_spmd(nc, in_maps, list(range(NCORES)))
    out = np.concatenate([res.results[c]["out"] for c in range(NCORES)], axis=0)
    return out


# revision 10
# speedup vs baseline: 1.1695x; 1.1695x over previous
"""BsplineKAN fused kernel for Trainium2 (8 NeuronCores, batch-sharded).

Math (per reference):
  basis = truncated in-place Cox-de Boor, degree 3, K=11 uniform knots on [0,1]
  out   = LN(einsum('bik,oik->bo', basis, cp) + x @ W.T + b) * gamma + beta

Closed form used here (u = 11*x, s_m = relu(u - m)):
  basis_k (k=0..7) = (1/6) * [s_k^3 - 4 s_{k+1}^3 + 6 s_{k+2}^3 - 4 s_{k+3}^3 + s_{k+4}^3]
  basis_8  = (1/2) * [s_8^2 - 3 s_9^2 + 3 s_10^2]
  basis_9  = s_9 - 2 s_10
  basis_10 = (sign(u - 10) + 1) / 2
The linear layer is fused as a 12th basis column (feature = x, weights = W),
the +1/2 constant of basis_10 and the bias b are folded into a single K=1
ones-row matmul. Scale factors (1/6, 1/2) are folded into the control-point
matrix on the host. The big contraction (K = 12*1024) runs on the PE in bf16;
the basis is combined on-device in fp32 (the relu^3 terms reach ~1300 while
basis values are <1, so pre-combine bf16 quantization would be catastrophic).
"""

import functools
import numpy as np
import ml_dtypes

BATCH = 16384
INF = 1024
OUTF = 1024
NCORES = 8
BC = BATCH // NCORES        # 2048 batch rows per core
BMS = 512                   # batch-macro size (basis slice width)
NBM = BC // BMS             # 4 macros
IB = INF // 128             # 8 i-blocks
CPI = 12                    # feature rows per i (11 spline cols + x)
NCHUNK = IB * CPI           # 96 contraction chunks of 128
EPS = 1e-5


@functools.lru_cache(maxsize=1)
def _build_nc():
    import concourse.bass as bass
    import concourse.mybir as mybir
    import concourse.tile as tile
    from concourse import bacc

    f32 = mybir.dt.float32
    bf16 = mybir.dt.bfloat16
    AF = mybir.ActivationFunctionType
    OP = mybir.AluOpType

    nc = bacc.Bacc("TRN2", target_bir_lowering=False, debug=False)
    xT = nc.dram_tensor("xT", [INF, BC], f32, kind="ExternalInput").ap()
    cpb = nc.dram_tensor("cpb", [NCHUNK * 128, OUTF], bf16, kind="ExternalInput").ap()
    brow = nc.dram_tensor("brow", [1, OUTF], f32, kind="ExternalInput").ap()
    gam = nc.dram_tensor("gam", [1, OUTF], f32, kind="ExternalInput").ap()
    bet = nc.dram_tensor("bet", [1, OUTF], f32, kind="ExternalInput").ap()
    out_d = nc.dram_tensor("out", [BC, OUTF], f32, kind="ExternalOutput").ap()

    with tile.TileContext(nc) as tc:
        from contextlib import ExitStack
        with ExitStack() as ctx:
            ep = ctx.enter_context
            consts = ep(tc.tile_pool(name="consts", bufs=1))
            xpool = ep(tc.tile_pool(name="xp", bufs=2))
            spool = ep(tc.tile_pool(name="sp", bufs=1))
            s2pool = ep(tc.tile_pool(name="s2p", bufs=4))
            s3pool = ep(tc.tile_pool(name="s3p", bufs=1))
            tpool = ep(tc.tile_pool(name="tp", bufs=6))
            bpool = ep(tc.tile_pool(name="bp", bufs=2))
            wpool = ep(tc.tile_pool(name="wp", bufs=3))
            zpool = ep(tc.tile_pool(name="zp", bufs=2))
            stpool = ep(tc.tile_pool(name="stp", bufs=2))
            ypool = ep(tc.tile_pool(name="yp", bufs=2))
            ppool = ep(tc.tile_pool(name="pp", bufs=8, space="PSUM"))

            gamma_t = consts.tile([128, OUTF], f32)
            nc.sync.dma_start(out=gamma_t, in_=gam.partition_broadcast(128))
            beta_t = consts.tile([128, OUTF], f32)
            nc.sync.dma_start(out=beta_t, in_=bet.partition_broadcast(128))
            brow_t = consts.tile([1, OUTF], f32)
            nc.sync.dma_start(out=brow_t, in_=brow)
            ones_t = consts.tile([1, 128], f32)
            nc.vector.memset(ones_t, 1.0)
            # col 0: eps for LN; cols 1..11: -m ACT bias constants
            mconst = consts.tile([128, 12], f32)
            nc.vector.memset(mconst[:, 0:1], EPS)
            for m in range(11):
                nc.vector.memset(mconst[:, m + 1:m + 2], -float(m))

            for bm in range(NBM):
                psums = [[ppool.tile([128, 512], f32, name="psum", tag="psum")
                          for _ in range(2)]
                         for _ in range(4)]
                for ib in range(IB):
                    xt = xpool.tile([128, BMS], f32)
                    nc.sync.dma_start(
                        out=xt, in_=xT[ib * 128:(ib + 1) * 128,
                                       bm * BMS:(bm + 1) * BMS])
                    # s[:, m, :] = relu(11*x - m), fp32
                    st = spool.tile([128, 11, BMS], f32)
                    for m in range(11):
                        nc.scalar.activation(out=st[:, m, :], in_=xt,
                                             func=AF.Relu,
                                             bias=mconst[:, m + 1:m + 2],
                                             scale=11.0)
                    # s3[:, m, :] = s^3 via rotating s2 tiles
                    s3t = s3pool.tile([128, 11, BMS], f32)
                    for m in range(11):
                        s2m = s2pool.tile([128, BMS], f32, name="s2m")
                        nc.scalar.activation(out=s2m, in_=st[:, m, :],
                                             func=AF.Square)
                        nc.gpsimd.tensor_mul(s3t[:, m, :], s2m, st[:, m, :])

                    bsl = bpool.tile([128, CPI, BMS], bf16)
                    # cubic cols: 4th difference of s^3 (1/6 folded in cpb)
                    for k in range(8):
                        t1 = tpool.tile([128, BMS], f32, name="tt", tag="tt")
                        nc.vector.scalar_tensor_tensor(
                            out=t1, in0=s3t[:, k + 1, :], scalar=-4.0,
                            in1=s3t[:, k, :], op0=OP.mult, op1=OP.add)
                        t2 = tpool.tile([128, BMS], f32, name="tt", tag="tt")
                        nc.vector.scalar_tensor_tensor(
                            out=t2, in0=s3t[:, k + 2, :], scalar=6.0, in1=t1,
                            op0=OP.mult, op1=OP.add)
                        if k < 7:
                            t3 = tpool.tile([128, BMS], f32, name="tt",
                                            tag="tt")
                            nc.vector.scalar_tensor_tensor(
                                out=t3, in0=s3t[:, k + 3, :], scalar=-4.0,
                                in1=t2, op0=OP.mult, op1=OP.add)
                            nc.vector.scalar_tensor_tensor(
                                out=bsl[:, k, :], in0=s3t[:, k + 4, :],
                                scalar=1.0, in1=t3, op0=OP.mult, op1=OP.add)
                        else:
                            nc.vector.scalar_tensor_tensor(
                                out=bsl[:, k, :], in0=s3t[:, 10, :],
                                scalar=-4.0, in1=t2, op0=OP.mult, op1=OP.add)
                    # quadratic col 8 = s8^2 - 3 s9^2 + 3 s10^2 (1/2 folded)
                    q8 = tpool.tile([128, BMS], f32, name="tt", tag="tt")
                    nc.gpsimd.tensor_mul(q8, st[:, 8, :], st[:, 8, :])
                    q9 = tpool.tile([128, BMS], f32, name="tt", tag="tt")
                    nc.gpsimd.tensor_mul(q9, st[:, 9, :], st[:, 9, :])
                    q10 = tpool.tile([128, BMS], f32, name="tt", tag="tt")
                    nc.gpsimd.tensor_mul(q10, st[:, 10, :], st[:, 10, :])
                    qa = tpool.tile([128, BMS], f32, name="tt", tag="tt")
                    nc.vector.scalar_tensor_tensor(
                        out=qa, in0=q9, scalar=-3.0, in1=q8,
                        op0=OP.mult, op1=OP.add)
                    nc.vector.scalar_tensor_tensor(
                        out=bsl[:, 8, :], in0=q10, scalar=3.0, in1=qa,
                        op0=OP.mult, op1=OP.add)
                    # linear col 9 = s9 - 2 s10
                    nc.vector.scalar_tensor_tensor(
                        out=bsl[:, 9, :], in0=st[:, 10, :], scalar=-2.0,
                        in1=st[:, 9, :], op0=OP.mult, op1=OP.add)
                    # step col 10 as sign (affine fold in cpb + brow)
                    nc.scalar.activation(out=bsl[:, 10, :], in_=xt,
                                         func=AF.Sign,
                                         bias=mconst[:, 11:12], scale=11.0)
                    # linear-layer feature: x itself
                    nc.vector.tensor_copy(out=bsl[:, 11, :], in_=xt)

                    for c in range(CPI):
                        chunk = ib * CPI + c
                        wt = wpool.tile([128, OUTF], bf16)
                        nc.sync.dma_start(
                            out=wt,
                            in_=cpb[chunk * 128:(chunk + 1) * 128, :])
                        first = (ib == 0 and c == 0)
                        for bs_i in range(4):
                            lhsT = bsl[:, c, bs_i * 128:(bs_i + 1) * 128]
                            for oh in range(2):
                                nc.tensor.matmul(
                                    psums[bs_i][oh], lhsT,
                                    wt[:, oh * 512:(oh + 1) * 512],
                                    start=first, stop=False)

                # bias row (b + 0.5*sum_i cp[:,i,10]) via ones-row matmul
                for bs_i in range(4):
                    for oh in range(2):
                        nc.tensor.matmul(
                            psums[bs_i][oh], ones_t,
                            brow_t[:, oh * 512:(oh + 1) * 512],
                            start=False, stop=True)

                # LayerNorm epilogue
                for bs_i in range(4):
                    z = zpool.tile([128, OUTF], f32)
                    nc.scalar.copy(z[:, 0:512], psums[bs_i][0])
                    nc.scalar.copy(z[:, 512:1024], psums[bs_i][1])
                    stats = stpool.tile([128, 2, 6], f32)
                    nc.vector.bn_stats(out=stats[:, 0, :], in_=z[:, 0:512])
                    nc.vector.bn_stats(out=stats[:, 1, :], in_=z[:, 512:1024])
                    mvsi = stpool.tile([128, 4], f32)
                    nc.vector.bn_aggr(out=mvsi[:, 0:2], in_=stats)
                    nc.scalar.activation(out=mvsi[:, 2:3], in_=mvsi[:, 1:2],
                                         func=AF.Sqrt, bias=mconst[:, 0:1])
                    nc.vector.reciprocal(out=mvsi[:, 3:4], in_=mvsi[:, 2:3])
                    y = ypool.tile([128, OUTF], f32)
                    nc.vector.tensor_scalar(
                        out=y, in0=z, scalar1=mvsi[:, 0:1],
                        scalar2=mvsi[:, 3:4],
                        op0=OP.subtract, op1=OP.mult)
                    nc.gpsimd.tensor_mul(y, y, gamma_t)
                    nc.gpsimd.tensor_add(y, y, beta_t)
                    row = bm * BMS + bs_i * 128
                    nc.sync.dma_start(out=out_d[row:row + 128, :], in_=y)

    nc.compile()
    return nc


def _host_prep(x, control_points, W, b):
    """Build per-core inputs. cpb row (k*1024+i) holds the weights for
    feature (k, i); scale factors folded in."""
    cp64 = control_points.astype(np.float64)
    blocks = []
    for k in range(12):
        if k < 8:
            blk = cp64[:, :, k].T / 6.0
        elif k == 8:
            blk = cp64[:, :, 8].T / 2.0
        elif k == 9:
            blk = cp64[:, :, 9].T
        elif k == 10:
            blk = cp64[:, :, 10].T / 2.0
        else:
            blk = W.astype(np.float64).T
        blocks.append(blk)
    # device chunk order: chunk = ib*12 + k  (i-block major, feature minor)
    kmaj = np.concatenate(blocks, axis=0).reshape(12, IB, 128, OUTF)
    cpb = np.ascontiguousarray(
        kmaj.transpose(1, 0, 2, 3).reshape(12 * INF, OUTF)
    ).astype(ml_dtypes.bfloat16)
    brow = (b.astype(np.float64)
            + 0.5 * cp64[:, :, 10].sum(axis=1)).astype(np.float32)[None, :]
    xT = np.ascontiguousarray(x.T)  # [INF, BATCH]
    return xT, cpb, brow


def kernel(x, control_points, W, b, gamma, beta):
    from concourse.bass_utils import run_bass_kernel_spmd

    xT, cpb, brow = _host_prep(x, control_points, W, b)
    gam = np.ascontiguousarray(gamma.astype(np.float32))[None, :]
    bet = np.ascontiguousarray(beta.astype(np.float32))[None, :]

    nc = _build_nc()
    in_maps = []
    for c in range(NCORES):
        in_maps.append({
            "xT": np.ascontiguousarray(xT[:, c * BC:(c + 1) * BC]),
            "cpb": cpb,
            "brow": brow,
            "gam": gam,
            "bet": bet,
        })
    res = run_bass_kernel_spmd(nc, in_maps, list(range(NCORES)))
    out = np.concatenate([res.results[c]["out"] for c in range(NCORES)], axis=0)
    return out


# revision 11
# speedup vs baseline: 1.5851x; 1.3554x over previous
"""BsplineKAN fused kernel for Trainium2 (8 NeuronCores, batch-sharded).

Math (per reference):
  basis = truncated in-place Cox-de Boor, degree 3, K=11 uniform knots on [0,1]
  out   = LN(einsum('bik,oik->bo', basis, cp) + x @ W.T + b) * gamma + beta

Closed form used here (u = 11*x, s_m = relu(u - m)):
  basis_k (k=0..7) = (1/6) * [s_k^3 - 4 s_{k+1}^3 + 6 s_{k+2}^3 - 4 s_{k+3}^3 + s_{k+4}^3]
  basis_8  = (1/2) * [s_8^2 - 3 s_9^2 + 3 s_10^2]
  basis_9  = s_9 - 2 s_10
  basis_10 = (sign(u - 10) + 1) / 2
The linear layer is fused as a 12th basis column (feature = x, weights = W),
the +1/2 constant of basis_10 and the bias b are folded into a single K=1
ones-row matmul. Scale factors (1/6, 1/2) are folded into the control-point
matrix on the host. The big contraction (K = 12*1024) runs on the PE in bf16;
the basis is combined on-device in fp32 (the relu^3 terms reach ~1300 while
basis values are <1, so pre-combine bf16 quantization would be catastrophic).
"""

import functools
import numpy as np
import ml_dtypes

BATCH = 16384
INF = 1024
OUTF = 1024
NCORES = 8
BC = BATCH // NCORES        # 2048 batch rows per core
BMS = 512                   # batch-macro size (basis slice width)
NBM = BC // BMS             # 4 macros
IB = INF // 128             # 8 i-blocks
CPI = 12                    # feature rows per i (11 spline cols + x)
NCHUNK = IB * CPI           # 96 contraction chunks of 128
EPS = 1e-5


@functools.lru_cache(maxsize=1)
def _build_nc():
    import concourse.bass as bass
    import concourse.mybir as mybir
    import concourse.tile as tile
    from concourse import bacc

    f32 = mybir.dt.float32
    bf16 = mybir.dt.bfloat16
    AF = mybir.ActivationFunctionType
    OP = mybir.AluOpType

    nc = bacc.Bacc("TRN2", target_bir_lowering=False, debug=False)
    xT = nc.dram_tensor("xT", [INF, BC], f32, kind="ExternalInput").ap()
    cpb = nc.dram_tensor("cpb", [NCHUNK * 128, OUTF], bf16, kind="ExternalInput").ap()
    brow = nc.dram_tensor("brow", [1, OUTF], f32, kind="ExternalInput").ap()
    gam = nc.dram_tensor("gam", [1, OUTF], f32, kind="ExternalInput").ap()
    bet = nc.dram_tensor("bet", [1, OUTF], f32, kind="ExternalInput").ap()
    out_d = nc.dram_tensor("out", [BC, OUTF], f32, kind="ExternalOutput").ap()

    with tile.TileContext(nc) as tc:
        from contextlib import ExitStack
        with ExitStack() as ctx:
            ep = ctx.enter_context
            consts = ep(tc.tile_pool(name="consts", bufs=1))
            xpool = ep(tc.tile_pool(name="xp", bufs=2))
            spool = ep(tc.tile_pool(name="sp", bufs=7))
            s2pool = ep(tc.tile_pool(name="s2p", bufs=5))
            s3pool = ep(tc.tile_pool(name="s3p", bufs=8))
            tpool = ep(tc.tile_pool(name="tp", bufs=5))
            bpool = ep(tc.tile_pool(name="bp", bufs=2))
            wpool = ep(tc.tile_pool(name="wp", bufs=4))
            zpool = ep(tc.tile_pool(name="zp", bufs=2))
            stpool = ep(tc.tile_pool(name="stp", bufs=2))
            ypool = ep(tc.tile_pool(name="yp", bufs=2))
            ppool = ep(tc.tile_pool(name="pp", bufs=8, space="PSUM"))

            gamma_t = consts.tile([128, OUTF], f32)
            nc.sync.dma_start(out=gamma_t, in_=gam.partition_broadcast(128))
            beta_t = consts.tile([128, OUTF], f32)
            nc.sync.dma_start(out=beta_t, in_=bet.partition_broadcast(128))
            brow_t = consts.tile([1, OUTF], f32)
            nc.sync.dma_start(out=brow_t, in_=brow)
            ones_t = consts.tile([1, 128], f32)
            nc.vector.memset(ones_t, 1.0)
            # col 0: eps for LN; cols 1..11: -m ACT bias constants
            mconst = consts.tile([128, 12], f32)
            nc.vector.memset(mconst[:, 0:1], EPS)
            for m in range(11):
                nc.vector.memset(mconst[:, m + 1:m + 2], -float(m))

            for bm in range(NBM):
                psums = [[ppool.tile([128, 512], f32, name="psum", tag="psum")
                          for _ in range(2)]
                         for _ in range(4)]
                for ib in range(IB):
                    xt = xpool.tile([128, BMS], f32)
                    nc.sync.dma_start(
                        out=xt, in_=xT[ib * 128:(ib + 1) * 128,
                                       bm * BMS:(bm + 1) * BMS])
                    # rotating tiles: s_m = relu(11x-m); s2 = s^2 (ACT);
                    # s3 = s^2 * s (GPSIMD)
                    s_l, s2_l, s3_l = [], [], []
                    for m in range(11):
                        sm = spool.tile([128, BMS], f32, name="sm", tag="sm")
                        nc.scalar.activation(out=sm, in_=xt, func=AF.Relu,
                                             bias=mconst[:, m + 1:m + 2],
                                             scale=11.0)
                        s2m = s2pool.tile([128, BMS], f32, name="s2m",
                                          tag="s2m")
                        nc.scalar.activation(out=s2m, in_=sm, func=AF.Square)
                        s3m = s3pool.tile([128, BMS], f32, name="s3m",
                                          tag="s3m")
                        nc.gpsimd.tensor_mul(s3m, s2m, sm)
                        s_l.append(sm)
                        s2_l.append(s2m)
                        s3_l.append(s3m)

                    bsl = bpool.tile([128, CPI, BMS], bf16)
                    # cubic cols: 4th difference of s^3 (1/6 folded in cpb)
                    for k in range(8):
                        t1 = tpool.tile([128, BMS], f32, name="tt", tag="tt")
                        nc.vector.scalar_tensor_tensor(
                            out=t1, in0=s3_l[k + 1], scalar=-4.0,
                            in1=s3_l[k], op0=OP.mult, op1=OP.add)
                        t2 = tpool.tile([128, BMS], f32, name="tt", tag="tt")
                        nc.vector.scalar_tensor_tensor(
                            out=t2, in0=s3_l[k + 2], scalar=6.0, in1=t1,
                            op0=OP.mult, op1=OP.add)
                        if k < 7:
                            t3 = tpool.tile([128, BMS], f32, name="tt",
                                            tag="tt")
                            nc.vector.scalar_tensor_tensor(
                                out=t3, in0=s3_l[k + 3], scalar=-4.0,
                                in1=t2, op0=OP.mult, op1=OP.add)
                            nc.vector.scalar_tensor_tensor(
                                out=bsl[:, k, :], in0=s3_l[k + 4],
                                scalar=1.0, in1=t3, op0=OP.mult, op1=OP.add)
                        else:
                            nc.vector.scalar_tensor_tensor(
                                out=bsl[:, k, :], in0=s3_l[10],
                                scalar=-4.0, in1=t2, op0=OP.mult, op1=OP.add)
                    # quadratic col 8 = s8^2 - 3 s9^2 + 3 s10^2 (1/2 folded)
                    qa = tpool.tile([128, BMS], f32, name="tt", tag="tt")
                    nc.vector.scalar_tensor_tensor(
                        out=qa, in0=s2_l[9], scalar=-3.0, in1=s2_l[8],
                        op0=OP.mult, op1=OP.add)
                    nc.vector.scalar_tensor_tensor(
                        out=bsl[:, 8, :], in0=s2_l[10], scalar=3.0, in1=qa,
                        op0=OP.mult, op1=OP.add)
                    # linear col 9 = s9 - 2 s10
                    nc.vector.scalar_tensor_tensor(
                        out=bsl[:, 9, :], in0=s_l[10], scalar=-2.0,
                        in1=s_l[9], op0=OP.mult, op1=OP.add)
                    # step col 10 as sign (affine fold in cpb + brow)
                    nc.scalar.activation(out=bsl[:, 10, :], in_=xt,
                                         func=AF.Sign,
                                         bias=mconst[:, 11:12], scale=11.0)
                    # linear-layer feature: x itself
                    nc.scalar.copy(bsl[:, 11, :], xt)

                    for c in range(CPI):
                        chunk = ib * CPI + c
                        wt = wpool.tile([128, OUTF], bf16)
                        nc.sync.dma_start(
                            out=wt,
                            in_=cpb[chunk * 128:(chunk + 1) * 128, :])
                        first = (ib == 0 and c == 0)
                        for bs_i in range(4):
                            lhsT = bsl[:, c, bs_i * 128:(bs_i + 1) * 128]
                            for oh in range(2):
                                nc.tensor.matmul(
                                    psums[bs_i][oh], lhsT,
                                    wt[:, oh * 512:(oh + 1) * 512],
                                    start=first, stop=False)

                # bias row (b + 0.5*sum_i cp[:,i,10]) via ones-row matmul
                for bs_i in range(4):
                    for oh in range(2):
                        nc.tensor.matmul(
                            psums[bs_i][oh], ones_t,
                            brow_t[:, oh * 512:(oh + 1) * 512],
                            start=False, stop=True)

                # LayerNorm epilogue
                for bs_i in range(4):
                    z = zpool.tile([128, OUTF], f32)
                    nc.scalar.copy(z[:, 0:512], psums[bs_i][0])
                    nc.scalar.copy(z[:, 512:1024], psums[bs_i][1])
                    stt = stpool.tile([128, 16], f32, name="stt2", tag="stt2")
                    stats = stt[:, 0:12].rearrange("p (g s) -> p g s", g=2)
                    mvsi = stt[:, 12:16]
                    nc.vector.bn_stats(out=stats[:, 0, :], in_=z[:, 0:512])
                    nc.vector.bn_stats(out=stats[:, 1, :], in_=z[:, 512:1024])
                    nc.vector.bn_aggr(out=mvsi[:, 0:2], in_=stats)
                    nc.scalar.activation(out=mvsi[:, 2:3], in_=mvsi[:, 1:2],
                                         func=AF.Sqrt, bias=mconst[:, 0:1])
                    nc.vector.reciprocal(out=mvsi[:, 3:4], in_=mvsi[:, 2:3])
                    y = ypool.tile([128, OUTF], f32)
                    nc.vector.tensor_scalar(
                        out=y, in0=z, scalar1=mvsi[:, 0:1],
                        scalar2=mvsi[:, 3:4],
                        op0=OP.subtract, op1=OP.mult)
                    nc.gpsimd.tensor_mul(y, y, gamma_t)
                    nc.gpsimd.tensor_add(y, y, beta_t)
                    row = bm * BMS + bs_i * 128
                    nc.sync.dma_start(out=out_d[row:row + 128, :], in_=y)

    nc.compile()
    return nc


def _host_prep(x, control_points, W, b):
    """Build per-core inputs. cpb row (k*1024+i) holds the weights for
    feature (k, i); scale factors folded in."""
    cp64 = control_points.astype(np.float64)
    blocks = []
    for k in range(12):
        if k < 8:
            blk = cp64[:, :, k].T / 6.0
        elif k == 8:
            blk = cp64[:, :, 8].T / 2.0
        elif k == 9:
            blk = cp64[:, :, 9].T
        elif k == 10:
            blk = cp64[:, :, 10].T / 2.0
        else:
            blk = W.astype(np.float64).T
        blocks.append(blk)
    # device chunk order: chunk = ib*12 + k  (i-block major, feature minor)
    kmaj = np.concatenate(blocks, axis=0).reshape(12, IB, 128, OUTF)
    cpb = np.ascontiguousarray(
        kmaj.transpose(1, 0, 2, 3).reshape(12 * INF, OUTF)
    ).astype(ml_dtypes.bfloat16)
    brow = (b.astype(np.float64)
            + 0.5 * cp64[:, :, 10].sum(axis=1)).astype(np.float32)[None, :]
    xT = np.ascontiguousarray(x.T)  # [INF, BATCH]
    return xT, cpb, brow


def kernel(x, control_points, W, b, gamma, beta):
    from concourse.bass_utils import run_bass_kernel_spmd

    xT, cpb, brow = _host_prep(x, control_points, W, b)
    gam = np.ascontiguousarray(gamma.astype(np.float32))[None, :]
    bet = np.ascontiguousarray(beta.astype(np.float32))[None, :]

    nc = _build_nc()
    in_maps = []
    for c in range(NCORES):
        in_maps.append({
            "xT": np.ascontiguousarray(xT[:, c * BC:(c + 1) * BC]),
            "cpb": cpb,
            "brow": brow,
            "gam": gam,
            "bet": bet,
        })
    res = run_bass_kernel_spmd(nc, in_maps, list(range(NCORES)))
    out = np.concatenate([res.results[c]["out"] for c in range(NCORES)], axis=0)
    return out
